# revision 1
# baseline (speedup 1.0000x reference)
"""Trainium2 Bass kernel for MetapathAggreLayer.

Computes, per node n:
    score[n, c] = sum_h hs[c, n, h] * v[c, h]        (c = 8 channels)
    att = softmax(score, axis=c)
    out[n, h]   = sum_c att[n, c] * hs[c, n, h]

Sharding: data-parallel over nodes across 8 NeuronCores (50000 nodes each).
meta_att_vec is replicated (pre-broadcast across partitions on the host).

On-chip layout: nodes on the partition axis (128/tile), (channel, hid) on the
free axis. Per 256-node macro-tile:
  - DVE: hs * v_bcast multiply, segmented reduce -> score[n, c], softmax sums,
         reciprocal, final cross-channel reduce
  - ACT: exp, and the 16 per-(group, channel) weighted scalings att[n,c]*hs_c
  - DMA via HWDGE (nc.sync)
"""

import sys

if "/opt/trn_rl_repo" not in sys.path:
    sys.path.insert(0, "/opt/trn_rl_repo")

import numpy as np

NCH = 8
NNODE = 400000
NHID = 128
NCORES = 8
NPC = NNODE // NCORES  # 50000 nodes per core
P = 128
NG = 2  # node groups (of 128) per macro-tile
FULL_ITERS = NPC // (NG * P)  # 195
TAIL = NPC - FULL_ITERS * NG * P  # 80

_cache = {}


def _build_program():
    import concourse.bacc as bacc
    import concourse.tile as tile
    import concourse.mybir as mybir

    dt = mybir.dt
    AX = mybir.AxisListType.X
    AF = mybir.ActivationFunctionType

    nc = bacc.Bacc("TRN2", target_bir_lowering=False, debug=False)
    hs_d = nc.dram_tensor("hs", [NCH, NPC, NHID], dt.float32, kind="ExternalInput").ap()
    vb_d = nc.dram_tensor("vb", [P, NCH, NHID], dt.float32, kind="ExternalInput").ap()
    id_d = nc.dram_tensor("ident", [P, P], dt.float16, kind="ExternalInput").ap()
    out_d = nc.dram_tensor("out", [NPC, NHID], dt.float32, kind="ExternalOutput").ap()

    with tile.TileContext(nc) as tc:
        with (
            tc.tile_pool(name="const", bufs=1) as cpool,
            tc.tile_pool(name="hs", bufs=5) as hpool,
            tc.tile_pool(name="prod", bufs=4) as ppool,
            tc.tile_pool(name="wsum", bufs=4) as wpool,
            tc.tile_pool(name="small", bufs=8) as spool,
            tc.tile_pool(name="outp", bufs=6) as opool,
            tc.tile_pool(name="ps", bufs=4, space="PSUM") as pspool,
        ):
            vb = cpool.tile([P, NCH, NHID], dt.float32)
            nc.sync.dma_start(vb[:], vb_d[:])
            I16 = cpool.tile([P, P], dt.float16)
            nc.sync.dma_start(I16[:], id_d[:])
            bconst = cpool.tile([P, 1], dt.float32)
            nc.vector.memset(bconst[:], -5.545177444479562)

            def body(base, ng, p):
                n = ng * p
                hs_t = hpool.tile([P, ng, NCH, NHID], dt.float32, tag="hs")
                for g in range(ng):
                    nc.sync.dma_start(
                        hs_t[:p, g],
                        hs_d[:, base + g * p : base + (g + 1) * p, :].rearrange(
                            "c p h -> p c h"
                        ),
                    )

                # score[n, (g c)] = sum_h hs * v
                prod = ppool.tile([P, ng, NCH, NHID], dt.float32, tag="prod")
                vbb = vb[:p].unsqueeze(1).broadcast_to([p, ng, NCH, NHID])
                nc.vector.tensor_mul(prod[:p], hs_t[:p], vbb)
                score = spool.tile([P, ng, NCH], dt.float32, tag="score")
                nc.vector.reduce_sum(score[:p], prod[:p], axis=AX)

                # softmax over c (8 wide); scores are O(10), exp is safe in fp32.
                # exp WITHOUT accum_out (the accumulator read costs an extra
                # 279ns ACT instr); channel sum on DVE instead (cheap 8-wide)
                # biased exp keeps unnormalized e*hs within fp16 range;
                # softmax is shift-invariant so the bias cancels in r
                e = spool.tile([P, ng, NCH], dt.float32, tag="e")
                nc.scalar.activation(e[:p], score[:p], AF.Exp, bias=bconst[:p])
                s = spool.tile([P, ng], dt.float32, tag="s")
                nc.vector.reduce_sum(s[:p], e[:p], axis=AX)
                r = spool.tile([P, ng], dt.float32, tag="r")
                nc.vector.reciprocal(r[:p], s[:p])

                # weighted sum with UNNORMALIZED e (normalization folded
                # into the scaled PSUM->SBUF copy below); fp16 out, split
                # DVE/ACT to balance engines; channel sum on PE via identity
                # matmuls accumulating in PSUM
                wsum = wpool.tile([P, ng, NCH, NHID], dt.float16, tag="wsum")
                for g in range(ng):
                    for c in range(NCH):
                        if c <= 1:
                            nc.vector.tensor_scalar_mul(
                                wsum[:p, g, c, :],
                                hs_t[:p, g, c, :],
                                e[:p, g, c : c + 1],
                            )
                        else:
                            nc.scalar.mul(
                                wsum[:p, g, c, :],
                                hs_t[:p, g, c, :],
                                e[:p, g, c : c + 1],
                            )
                out_t = opool.tile([P, ng, NHID], dt.float32, tag="out")
                for g in range(ng):
                    ops = pspool.tile([P, NHID], dt.float32, tag=f"ops{g}")
                    for c in range(NCH):
                        nc.tensor.matmul(
                            ops[:p, :],
                            I16[:p, :p],
                            wsum[:p, g, c, :],
                            start=(c == 0),
                            stop=(c == NCH - 1),
                        )
                    nc.scalar.mul(out_t[:p, g, :], ops[:p, :], r[:p, g : g + 1])

                nc.sync.dma_start(
                    out_d[base : base + n, :].rearrange("(g p) h -> p g h", p=p),
                    out_t[:p],
                )

            for i in range(FULL_ITERS):
                body(i * NG * P, NG, P)
            if TAIL:
                body(FULL_ITERS * NG * P, 1, TAIL)

    nc.compile()
    return nc


def _get_program():
    if "nc" not in _cache:
        _cache["nc"] = _build_program()
    return _cache["nc"]


def run(hs, meta_att_vec, trace=False):
    from concourse.bass_utils import run_bass_kernel_spmd

    nc = _get_program()
    hs = np.asarray(hs, dtype=np.float32)
    v = np.asarray(meta_att_vec, dtype=np.float32)
    vb = np.ascontiguousarray(
        np.broadcast_to(v.reshape(1, NCH, NHID), (P, NCH, NHID))
    )
    ident = np.eye(P, dtype=np.float16)
    in_maps = [
        {
            "hs": np.ascontiguousarray(hs[:, i * NPC : (i + 1) * NPC, :]),
            "vb": vb,
            "ident": ident,
        }
        for i in range(NCORES)
    ]
    res = run_bass_kernel_spmd(nc, in_maps, list(range(NCORES)), trace=trace)
    out = np.concatenate([res.results[i]["out"] for i in range(NCORES)], axis=0)
    return out, res


def kernel(hs, meta_att_vec, nnode=None):
    out, _ = run(hs, meta_att_vec, trace=False)
    return out



# revision 3
# speedup vs baseline: 1.5229x; 1.5229x over previous
"""Trainium2 Bass kernel for MetapathAggreLayer.

Computes, per node n:
    score[n, c] = sum_h hs[c, n, h] * v[c, h]        (c = 8 channels)
    att = softmax(score, axis=c)
    out[n, h]   = sum_c att[n, c] * hs[c, n, h]

Sharding: data-parallel over nodes across 8 NeuronCores (50000 nodes each).
meta_att_vec replicated (pre-broadcast on host). hs is converted to fp16 and
re-interleaved to [node, channel, hid] on the host so each 128-node DMA line
is 2048B contiguous; output returns fp16 and is cast to fp32 on the host.

Per 512-node macro-tile (4 groups of 128 nodes on partitions):
  DVE: hs*v multiply (fp16 2x), tree-add reduce L1-L3 + tensor_reduce,
       softmax sums/reciprocal, part of the per-channel e*hs scalings
  ACT: exp, the other e*hs scalings, r-scaled PSUM->SBUF output copies
  PE:  channel sum via 8 accumulating identity matmuls (512-wide rhs)
"""

import sys

if "/opt/trn_rl_repo" not in sys.path:
    sys.path.insert(0, "/opt/trn_rl_repo")

import numpy as np

NCH = 8
NNODE = 400000
NHID = 128
NCORES = 8
NPC = NNODE // NCORES  # 50000
P = 128
NG = 4
FULL_ITERS = NPC // (NG * P)  # 97
TAIL = NPC - FULL_ITERS * NG * P  # 336 = 3 * 112

# wsum engine split: (g, c) pairs 0..31 -> engine
N_ACT_WSUM = 15  # rest go to DVE

_cache = {}


def _build_program():
    import concourse.bacc as bacc
    import concourse.tile as tile
    import concourse.mybir as mybir

    dt = mybir.dt
    AX = mybir.AxisListType
    AF = mybir.ActivationFunctionType

    nc = bacc.Bacc("TRN2", target_bir_lowering=False, debug=False)
    hs_d = nc.dram_tensor("hs", [NPC, NCH, NHID], dt.float16, kind="ExternalInput").ap()
    vb_d = nc.dram_tensor("vb", [P, NCH, NHID], dt.float16, kind="ExternalInput").ap()
    id_d = nc.dram_tensor("ident", [P, P], dt.float16, kind="ExternalInput").ap()
    out_d = nc.dram_tensor("out", [NPC, NHID], dt.float16, kind="ExternalOutput").ap()

    with tile.TileContext(nc) as tc:
        with (
            tc.tile_pool(name="const", bufs=1) as cpool,
            tc.tile_pool(name="hs", bufs=3) as hpool,
            tc.tile_pool(name="prod", bufs=2) as ppool,
            tc.tile_pool(name="tree", bufs=2) as tpool,
            tc.tile_pool(name="wsum", bufs=2) as wpool,
            tc.tile_pool(name="small", bufs=6) as spool,
            tc.tile_pool(name="outp", bufs=4) as opool,
            tc.tile_pool(name="ps", bufs=2, space="PSUM") as pspool,
        ):
            vb = cpool.tile([P, NCH, NHID], dt.float16)
            nc.sync.dma_start(vb[:], vb_d[:])
            I16 = cpool.tile([P, P], dt.float16)
            nc.sync.dma_start(I16[:], id_d[:])
            bconst = cpool.tile([P, 1], dt.float32)
            nc.vector.memset(bconst[:], -5.545177444479562)

            def body(base, ng, p):
                n = ng * p
                hs_t = hpool.tile([P, NG, NCH, NHID], dt.float16, tag="hs")
                for g in range(ng):
                    nc.sync.dma_start(
                        hs_t[:p, g],
                        hs_d[base + g * p : base + (g + 1) * p],
                    )

                # P1: prod = hs * v (fp16, 2x), then tree-reduce over h
                prod = ppool.tile([P, NG, NCH, NHID], dt.float16, tag="prod")
                vbB = vb[:p].unsqueeze(1).broadcast_to([p, ng, NCH, NHID])
                nc.vector.tensor_mul(prod[:p, 0:ng], hs_t[:p, 0:ng], vbB)

                t1 = tpool.tile([P, NG, NCH, 64], dt.float16, tag="t1")
                nc.vector.tensor_add(
                    t1[:p, 0:ng], prod[:p, 0:ng, :, 0:64], prod[:p, 0:ng, :, 64:128]
                )
                t2 = tpool.tile([P, NG, NCH, 32], dt.float16, tag="t2")
                nc.vector.tensor_add(
                    t2[:p, 0:ng], t1[:p, 0:ng, :, 0:32], t1[:p, 0:ng, :, 32:64]
                )
                t3 = tpool.tile([P, NG, NCH, 16], dt.float16, tag="t3")
                nc.vector.tensor_add(
                    t3[:p, 0:ng], t2[:p, 0:ng, :, 0:16], t2[:p, 0:ng, :, 16:32]
                )
                score = spool.tile([P, NG, NCH], dt.float32, tag="score")
                nc.vector.reduce_sum(score[:p, 0:ng], t3[:p, 0:ng], axis=AX.X)

                # softmax over c; biased exp keeps e in safe range, the bias
                # cancels through r = 1/sum(e)
                e32 = spool.tile([P, NG, NCH], dt.float32, tag="e32")
                nc.scalar.activation(
                    e32[:p, 0:ng], score[:p, 0:ng], AF.Exp, bias=bconst[:p]
                )
                s32 = spool.tile([P, NG], dt.float32, tag="s32")
                nc.vector.reduce_sum(s32[:p, 0:ng], e32[:p, 0:ng], axis=AX.X)
                r32 = spool.tile([P, NG], dt.float32, tag="r32")
                nc.vector.reciprocal(r32[:p, 0:ng], s32[:p, 0:ng])

                # P2: wsum[c][g] = e[g, c] * hs[g, c] (fp16), split ACT/DVE.
                # wsum layout [P, c, g, h] so the per-channel PE rhs slice is
                # contiguous (g, h).
                wsum = wpool.tile([P, NCH, NG, NHID], dt.float16, tag="wsum")
                k = 0
                for c in range(NCH):
                    for g in range(ng):
                        if k % 32 < N_ACT_WSUM:
                            nc.scalar.mul(
                                wsum[:p, c, g, :],
                                hs_t[:p, g, c, :],
                                e32[:p, g, c : c + 1],
                            )
                        else:
                            nc.vector.tensor_scalar_mul(
                                wsum[:p, c, g, :],
                                hs_t[:p, g, c, :],
                                e32[:p, g, c : c + 1],
                            )
                        k += 1

                # channel sum on PE: 8 accumulating identity matmuls,
                # rhs [p, ng*128]
                ps = pspool.tile([P, NG * NHID], dt.float32, tag="ps")
                for c in range(NCH):
                    nc.tensor.matmul(
                        ps[:p, 0 : ng * NHID],
                        I16[:p, :p],
                        wsum[:p, c, 0:ng].rearrange("p g h -> p (g h)"),
                        start=(c == 0),
                        stop=(c == NCH - 1),
                    )

                # normalize + copy PSUM->SBUF (fp16 out), then store
                out_t = opool.tile([P, NG, NHID], dt.float16, tag="out")
                for g in range(ng):
                    nc.scalar.mul(
                        out_t[:p, g],
                        ps[:p, g * NHID : (g + 1) * NHID],
                        r32[:p, g : g + 1],
                    )
                nc.sync.dma_start(
                    out_d[base : base + n].rearrange("(g pp) h -> pp g h", pp=p),
                    out_t[:p, 0:ng],
                )

            for i in range(FULL_ITERS):
                body(i * NG * P, NG, P)
            if TAIL:
                body(FULL_ITERS * NG * P, 3, TAIL // 3)

    nc.compile()
    return nc


def _get_program():
    if "nc" not in _cache:
        _cache["nc"] = _build_program()
    return _cache["nc"]


def _prep_inputs(hs, meta_att_vec):
    hs = np.asarray(hs)
    v = np.asarray(meta_att_vec, dtype=np.float32)
    vb = np.ascontiguousarray(
        np.broadcast_to(v.astype(np.float16).reshape(1, NCH, NHID), (P, NCH, NHID))
    )
    ident = np.eye(P, dtype=np.float16)
    # [c, n, h] -> per-core [n_slice, c, h] fp16
    hs16 = hs.astype(np.float16)
    in_maps = []
    for i in range(NCORES):
        sl = np.ascontiguousarray(
            hs16[:, i * NPC : (i + 1) * NPC, :].transpose(1, 0, 2)
        )
        in_maps.append({"hs": sl, "vb": vb, "ident": ident})
    return in_maps


def run(hs, meta_att_vec, trace=False):
    from concourse.bass_utils import run_bass_kernel_spmd

    nc = _get_program()
    in_maps = _prep_inputs(hs, meta_att_vec)
    res = run_bass_kernel_spmd(nc, in_maps, list(range(NCORES)), trace=trace)
    out = np.concatenate(
        [res.results[i]["out"].astype(np.float32) for i in range(NCORES)], axis=0
    )
    return out, res


def kernel(hs, meta_att_vec, nnode=None):
    out, _ = run(hs, meta_att_vec, trace=False)
    return out
